# revision 33
# baseline (speedup 1.0000x reference)
"""Pairwise KL divergence kernel for Trainium2, SPMD across 8 NeuronCores.

out[n, m] = sum_d a[n,d]*(log a[n,d] - log b[m,d])
          = ent[n] - (a @ log(b)^T)[n, m],  ent = rowsum(a * log a)

Sharding: a (and output rows) split 8 ways; b replicated.
Per core: a_shard (1024, 64), b (8192, 64) -> out_shard (1024, 8192).

The PE clock is pinned ~1.2 GHz in this environment (no HAM ramp). The
key trick: K=64 only needs half the 128-row PE array, so everything is
stacked on both partition halves and the two half-height matmuls execute
CONCURRENTLY on disjoint row groups (h0/h1), doubling GEMM throughput:
  - all input DMAs issued up front on the sync sequencer; chunk 0 of b is
    loaded/Ln'd in halves so the first transposes start ASAP.
  - b: lb = Ln(b) -> fp16 [ACT]; 32 double transposes ([128,128] input =
    two adjacent b tiles) put (tile 2k)^T on partitions 0-63 and
    (tile 2k+1)^T on partitions 64-127 of lbT; copies on DVE.
  - a: duplicated across both free halves, so 8 [128,128] transposes give
    aT stacked on both partition halves; ent chain (Ln(a) [ACT], a*la +
    reduce [DVE]) completes before the first evac.
  - main loop n-tile-major: per 2-bank psum group, bank0 = even b-tiles
    (partitions 0-63) and bank1 = odd (partitions 64-127, matmul derives
    tile_position=(64,0)); the two matmuls overlap on the array. Evac
    un-interleaves even/odd 128-col blocks in the write AP, fused with
    the entropy term (-psum + ent -> fp16), DVE bank0 / ACT bank1 in
    parallel; quarter-tile (512 KB) stores, finer on the last n-tile.
  - host upcasts fp16 -> fp32.

Precision: fp16 GEMM operands + fp16 output give ~5e-4 max rel err vs the
fp32 reference, well under the 2e-2 gate.
"""

import numpy as np

N, M, D = 8192, 8192, 64
NCORES = 8
NSHARD = N // NCORES          # 1024 rows of a per core
NT = NSHARD // 128            # 8 n-tiles per core
MT = M // 512                 # 16 m-tiles of 512
BT = M // 128                 # 64 b row-tiles to transpose
B_CHUNK = 16                  # b row-tiles per chunk (2048 rows)
N_CHUNKS = BT // B_CHUNK      # 4

MM_DTYPE = "fp16"
OUT_DTYPE = "fp16"

_CACHE = {}


def _build(mm_dtype, out_dtype):
    from contextlib import ExitStack

    import concourse.bacc as bacc_mod
    import concourse.bass as bass
    import concourse.mybir as mybir
    import concourse.tile as tile
    from concourse.masks import make_identity

    FP32 = mybir.dt.float32
    AF = mybir.ActivationFunctionType
    ALU = mybir.AluOpType
    AX = mybir.AxisListType

    DT_MM = {
        "fp16": mybir.dt.float16,
        "bf16": mybir.dt.bfloat16,
        "fp32": FP32,
        "fp32r": mybir.dt.float32r,
    }[mm_dtype]
    DT_OUT = {"fp16": mybir.dt.float16, "fp32": FP32}[out_dtype]
    two_byte = mm_dtype in ("fp16", "bf16")
    TP_DT = DT_MM if two_byte else FP32

    nc = bacc_mod.Bacc()
    a_d = nc.dram_tensor("a", [NSHARD, D], FP32, kind="ExternalInput")
    b_d = nc.dram_tensor("b", [M, D], FP32, kind="ExternalInput")
    out_d = nc.dram_tensor("out", [NSHARD, M], DT_OUT, kind="ExternalOutput")

    with tile.TileContext(nc) as tc, ExitStack() as ctx:
        consts = ctx.enter_context(tc.tile_pool(name="consts", bufs=1))
        apool = ctx.enter_context(tc.tile_pool(name="apool", bufs=1))
        bpool = ctx.enter_context(tc.tile_pool(name="bpool", bufs=N_CHUNKS))
        lbpool = ctx.enter_context(tc.tile_pool(name="lbpool", bufs=2))
        lbtp = ctx.enter_context(tc.tile_pool(name="lbtp", bufs=1))
        tpsum = ctx.enter_context(tc.tile_pool(name="tpsum", bufs=2, space="PSUM"))
        mmps = ctx.enter_context(tc.tile_pool(name="mmps", bufs=3, space="PSUM"))
        stage = ctx.enter_context(tc.tile_pool(name="stage", bufs=4))

        ident = consts.tile([128, 128], TP_DT)
        make_identity(nc, ident)
        # Dummy transpose so PE observes the gpsimd (ident) sem here: the
        # matmul/LDW struct only carries ONE sync wait, so later transposes
        # must each need at most one sem (codegen: "Too many sync waits").
        warm = tpsum.tile([128, 128], TP_DT, tag="tp")
        nc.tensor.transpose(warm, ident, ident)

        # -------- input DMAs, all issued up front on the sync sequencer ----
        # b chunk 0 goes first (in halves): the earliest PE work is its
        # transposes, so its data must land before a's
        b_r = b_d[:, :].rearrange("(t p) d -> p t d", p=128)
        b_nats = []
        for h in range(N_CHUNKS):
            b_nat = bpool.tile([128, B_CHUNK, D], FP32, tag="b_nat")
            b_nats.append(b_nat)
        half = B_CHUNK // 2
        nc.sync.dma_start(out=b_nats[0][:, :half, :], in_=b_r[:, :half, :])
        nc.sync.dma_start(out=b_nats[0][:, half:, :], in_=b_r[:, half:B_CHUNK, :])
        a_nat = apool.tile([128, NT, D], FP32)        # a row t*128+p at [p, t, :]
        nc.sync.dma_start(out=a_nat, in_=a_d[:, :].rearrange("(t p) d -> p t d", p=128))
        for h in range(1, N_CHUNKS):
            nc.sync.dma_start(
                out=b_nats[h], in_=b_r[:, h * B_CHUNK : (h + 1) * B_CHUNK, :]
            )

        # lbT[0:64, k, :] = (b tile 2k)^T, lbT[64:128, k, :] = (b tile 2k+1)^T
        # - each [128,128] PE transpose handles TWO b tiles at once
        lbT = lbtp.tile([128, BT // 2, 128], DT_MM)

        def b_chunk(h, split=1, copy_engs=None):
            lb = lbpool.tile([128, B_CHUNK, D], TP_DT, tag="lb")
            step = B_CHUNK // split
            for s in range(split):
                sl = slice(s * step, (s + 1) * step)
                nc.scalar.activation(lb[:, sl, :], b_nats[h][:, sl, :], AF.Ln)
            lb2 = lb.rearrange("p (k two) d -> p k (two d)", two=2)
            for gg in range(B_CHUNK // 4):
                k0 = h * (B_CHUNK // 2) + gg * 2      # global pair index
                tp = tpsum.tile([128, 2, 128], TP_DT, tag="tp")
                for j in range(2):
                    nc.tensor.transpose(tp[:, j], lb2[:, gg * 2 + j, :], ident)
                copy_engs[gg % len(copy_engs)](lbT[:, k0 : k0 + 2, :], tp)

        b_chunk(0, split=2, copy_engs=[nc.vector.tensor_copy])

        # ---------------- a prologue (after chunk 0 in the PE FIFO: its
        # data lands later than b0's, and the PE wait queue is only 4 deep)
        # a is duplicated across both free halves so one [128,128] transpose
        # yields aT stacked on BOTH partition halves (rows 64-127 serve the
        # odd-tile matmuls, whose lbT lives at partitions 64-127)
        a_mm2 = apool.tile([128, NT, 2, D], DT_MM)
        nc.vector.tensor_copy(a_mm2[:, :, 0, :], a_nat)
        nc.vector.tensor_copy(a_mm2[:, :, 1, :], a_nat)
        aT = apool.tile([128, NT, 128], DT_MM)        # [0:64]=aT_t, [64:128]=aT_t
        for g in range(2):
            tp = tpsum.tile([128, 4, 128], TP_DT, tag="tp")
            for j in range(4):
                t_ = g * 4 + j
                nc.tensor.transpose(tp[:, j], a_mm2[:, t_, :, :], ident)
            nc.vector.tensor_copy(aT[:, g * 4 : (g + 1) * 4, :], tp)

        # entropy chain (needed by the first evac, hides under the b loads)
        la = apool.tile([128, NT, D], FP32)
        nc.scalar.activation(la, a_nat, AF.Ln)
        prod = apool.tile([128, NT, D], FP32)
        nc.vector.tensor_mul(prod, a_nat, la)
        ent = apool.tile([128, NT], FP32)
        for t in range(NT):
            nc.vector.reduce_sum(ent[:, t : t + 1], prod[:, t, :], axis=AX.X)

        # chunks 1-3's copies land inside the main window where BOTH evac
        # engines are near-saturated - split them DVE/ACT to stay balanced
        for h in range(1, N_CHUNKS):
            b_chunk(h, copy_engs=[nc.vector.tensor_copy, nc.scalar.copy])

        # ---------------- main GEMM + fused evac ----------------
        out_r = out_d[:, :].rearrange("(t p) (c m) -> t p c m", p=128, m=512)
        for t in range(NT):
            out_sb = stage.tile([128, MT, 512], DT_OUT, tag="out_sb")
            lhsT = aT[:, t, :]
            ent_t = ent[:, t : t + 1]
            # quarter-tile stores start each piece earlier and smooth the
            # DMA stream; the last n-tile drains in even finer pieces
            dma_after = (
                {1: (0, 4), 3: (4, 8), 5: (8, 12), 6: (12, 14), 7: (14, 16)}
                if t == NT - 1
                else {1: (0, 4), 3: (4, 8), 5: (8, 12), 7: (12, 16)}
            )
            for g in range(MT // 2):
                ps = mmps.tile([128, 2, 512], FP32, tag="ps")
                # bank 0 <- even b-tiles (lbT partitions 0-63, aT rows 0-63),
                # bank 1 <- odd b-tiles (partitions 64-127; matmul derives
                # tile_position=(64,0) from the matching base partitions)
                nc.tensor.matmul(
                    ps[:, 0],
                    lhsT[0:64, :],
                    lbT[0:64, g * 4 : (g + 1) * 4, :],
                    start=True,
                    stop=True,
                )
                nc.tensor.matmul(
                    ps[:, 1],
                    lhsT[64:128, :],
                    lbT[64:128, g * 4 : (g + 1) * 4, :],
                    start=True,
                    stop=True,
                )
                # un-interleave even/odd 128-col blocks in the evac write AP
                dst = out_sb[:, g * 2 : (g + 1) * 2, :].rearrange(
                    "p c (u v i) -> p c u v i", u=2, v=2
                )
                src0 = ps[:, 0].rearrange("p (c u i) -> p c u i", c=2, u=2)
                src1 = ps[:, 1].rearrange("p (c u i) -> p c u i", c=2, u=2)
                # both engines drain every group in parallel (ring latency)
                nc.vector.tensor_scalar(
                    dst[:, :, :, 0, :], src0, -1.0, ent_t, ALU.mult, ALU.add
                )
                nc.scalar.activation(
                    dst[:, :, :, 1, :], src1, AF.Identity, bias=ent_t, scale=-1.0
                )
                if g in dma_after:
                    c0, c1 = dma_after[g]
                    nc.sync.dma_start(
                        out=out_r[t, :, c0:c1, :],
                        in_=out_sb[:, c0:c1, :],
                    )
    # bacc lowering: splits multi-sem waits onto event-semaphore/nop
    # instructions (HW allows one sync wait per engine instruction).
    nc.compile()
    return nc


def _run(a, b, trace=False):
    from concourse.bass_utils import run_bass_kernel_spmd

    key = (MM_DTYPE, OUT_DTYPE)
    if key not in _CACHE:
        _CACHE[key] = _build(*key)
    nc = _CACHE[key]
    a = np.ascontiguousarray(np.asarray(a, dtype=np.float32))
    b = np.ascontiguousarray(np.asarray(b, dtype=np.float32))
    in_maps = [
        {"a": a[i * NSHARD : (i + 1) * NSHARD], "b": b} for i in range(NCORES)
    ]
    res = run_bass_kernel_spmd(nc, in_maps, list(range(NCORES)), trace=trace)
    out = np.concatenate(
        [np.asarray(r["out"], dtype=np.float32) for r in res.results], axis=0
    )
    return out, res


def kernel(a, b):
    out, _ = _run(a, b, trace=False)
    return out
